# revision 3
# baseline (speedup 1.0000x reference)
"""Trainium2 Bass kernel for nn_Discriminator (attentional recent discriminator).

Math notes (derived from the module definition, hardcoded here):
  - The attention matmul result is deleted (torch sorts a size-1 dim, so the
    "top-5" indices are always 0); the output depends only on node_vec rows
    0 and N-1 of each batch element.
  - hidden_in rows 1..5 are all node_vec[:,0,:], so after the node MLP the
    five repeated u0 blocks contract against ta_w1 blocks 1..5; those blocks
    are PRE-SUMMED on the host, so stage 3 is 2 matmuls (last-block + summed).

Sharding: pure data parallel over batch, 32 batches/core on 8 cores.
Weights + tables replicated.

v5: the gpsimd ap_gather path (library load ~6us IRAM + ext-isa dispatch
quiesce put slot data at ~15us into the kernel) is gone.  The pe/emb
lookups index tiny replicated tables (200x8 / 200x16), so the host packs
the 64 needed slot vectors per core directly into a K-major bf16 tile
vTs [128, 2*64] (k-chunks 0,1 of the stage-1 contraction) shipped as an
input -- same class of input packing as the pre-summed ta_w1 blocks.
The large data-dependent gather (lstm rows, 512 wide out of a 6.5MB
per-core tensor) stays on device as an indirect DMA (64 descriptors,
one per out partition -- the HW-supported form), followed by 4 PE
transposes into K-major chunks 2..5.  Stage-1 slot matmuls issue as soon
as vTs+wst1 land (~8.5us), lstm matmuls right after the transposes.

Biases: node_b1 folded into K-row 240 (vTs carries a 1.0 there);
ta_b1/ff_b1/ff_b2/ts_b folded via an extra all-ones partition row on the
rhs tiles and a bias row in the packed weights; b2/ta_b2 applied with the
DVE tensor_scalar per-partition pointer.  All activations run on the DVE.

Precision: bf16 operands with fp32 PSUM accumulation.
"""

import ml_dtypes
import numpy as np

import concourse.bass as bass
import concourse.mybir as mybir
import concourse.tile as tile
from concourse import bacc
from concourse.bass import IndirectOffsetOnAxis
from concourse.bass_utils import run_bass_kernel_spmd

# problem constants (hardcoded per harness contract)
B, N, M = 256, 200, 200
EMB_DICT, EMB_DIM, POS_DIM, HID, LSTM_DIM, MAX_LEN, TOPK = 200, 16, 8, 32, 512, 200, 5
NODE_DIM = 2 * POS_DIM + 14 * EMB_DIM + LSTM_DIM  # 752

N_CORES = 8
NB = B // N_CORES  # 32 batches per core
NV = 2 * NB        # 64 node vectors per core (node 0 and node N-1)

SLOT_K = 256                  # pe0|pe1|emb*14|ones|pad, K-chunks 0,1
KDIM = SLOT_K + LSTM_DIM      # 768
NCHUNK = KDIM // 128          # 6

F32 = mybir.dt.float32
BF16 = mybir.dt.bfloat16
I32 = mybir.dt.int32
NP_BF16 = ml_dtypes.bfloat16

# wst1 (bf16) [128, 832]: stage-1 weight chunks at 128c, identity at 768
_C_IDENT = NCHUNK * 128  # 768
_WST1_COLS = 832

# wsm (bf16) [128, 417] column layout
_C_W2 = 0        # rows 0:128, node_w2
_C_TA1L = 32     # rows 0:33, ta_w1 block 0 (multiplies u_last), row 32 zero
_C_TA1R = 160    # rows 0:33, sum of ta_w1 blocks 1..5, row 32 = ta_b1
_C_TAW2 = 288    # rows 0:128
_C_FFW1 = 320    # rows 0:33, row 32 = ff_b1
_C_FFW2 = 384    # rows 0:65, row 64 = ff_b2
_C_TSW = 416     # rows 0:33, row 32 = ts_b
_WSM_COLS = 417

ADD = mybir.AluOpType.add
MAX = mybir.AluOpType.max


def _pos_encoding():
    pos = np.arange(MAX_LEN, dtype=np.float32)[:, None]
    div = np.exp(
        np.arange(0, POS_DIM, 2, dtype=np.float32) * (-np.log(10000.0) / POS_DIM)
    )
    pe = np.zeros((MAX_LEN, POS_DIM), np.float32)
    pe[:, 0::2] = np.sin(pos * div)
    pe[:, 1::2] = np.cos(pos * div)
    return pe


def build_nc():
    # Bacc (not plain Bass): its compile pass splits multi-wait sync into
    # InstEventSemaphore, which the walrus codegen requires (1 wait/inst).
    nc = bacc.Bacc(
        "TRN2",
        target_bir_lowering=False,
        debug=False,
        num_devices=N_CORES,
    )

    lstm = nc.dram_tensor("lstm", [NB * M, LSTM_DIM], BF16, kind="ExternalInput")
    vts_d = nc.dram_tensor("vts", [128, 2 * NV], BF16, kind="ExternalInput")
    li_d = nc.dram_tensor("li", [NV, 1], I32, kind="ExternalInput")
    wst1_d = nc.dram_tensor("wst1", [128, _WST1_COLS], BF16, kind="ExternalInput")
    wsm_d = nc.dram_tensor("wsm", [128, _WSM_COLS], BF16, kind="ExternalInput")
    wfc_d = nc.dram_tensor("wfc", [HID, 2], F32, kind="ExternalInput")

    out_d = nc.dram_tensor("out", [1, NB], F32, kind="ExternalOutput")

    with tile.TileContext(nc) as tc:
        with (
            tc.tile_pool(name="sb", bufs=1) as sb,
            tc.tile_pool(name="pst", bufs=3, space="PSUM") as pst,
            tc.tile_pool(name="ps", bufs=1, space="PSUM") as ps,
        ):
            # ---- input DMAs: li first (gates the lstm gather), then the
            # stage-1 operands, then the tail weights.
            li = sb.tile([NV, 1], I32, tag="li")
            # single_packet: 64x4B descriptors ride one packet on one SDMA
            # engine, so the completion sem fires once instead of trickling
            # across 16 engines (~0.4us earlier gather start).
            nc.sync.dma_start(li[:], li_d[:], single_packet=True)
            vTs = sb.tile([128, 2 * NV], BF16, tag="vTs")
            nc.sync.dma_start(vTs[:], vts_d[:])
            wst1 = sb.tile([128, _WST1_COLS], BF16, tag="wst1")
            nc.scalar.dma_start(wst1[:], wst1_d[:])
            wsm = sb.tile([128, _WSM_COLS], BF16, tag="wsm")
            nc.scalar.dma_start(wsm[:], wsm_d[:])
            wfc = sb.tile([HID, 2], F32, tag="wfc")
            nc.sync.dma_start(wfc[:], wfc_d[:])

            # ---- lstm gather: 64 descriptors on the dynamic queue ----
            nvL = sb.tile([NV, LSTM_DIM], BF16, tag="nvL")
            nc.gpsimd.indirect_dma_start(
                out=nvL[:], out_offset=None, in_=lstm[:],
                in_offset=IndirectOffsetOnAxis(ap=li[:, 0:1], axis=0),
            )

            # ---- ones rows for bias folding (off critical path) ----
            u = sb.tile([HID + 1, NV], BF16, tag="u")
            g2 = sb.tile([HID + 1, NB], BF16, tag="g2")
            g3 = sb.tile([2 * HID + 1, NB], BF16, tag="g3")
            g4 = sb.tile([HID + 1, NB], BF16, tag="g4")
            nc.vector.memset(u[HID : HID + 1, :], 1.0)
            nc.vector.memset(g2[HID : HID + 1, :], 1.0)
            nc.vector.memset(g3[2 * HID : 2 * HID + 1, :], 1.0)
            nc.vector.memset(g4[HID : HID + 1, :], 1.0)

            # ---- stage 1: h1 = relu(w1p.T @ v)  [128, NV]; b1 in K-row 240
            # slot chunks first (vTs lands early); lstm chunks after the
            # transposes fill vTl.
            h1p = ps.tile([128, NV], F32, tag="h1p")
            nc.tensor.matmul(
                h1p[:], lhsT=wst1[:, bass.ts(0, 128)], rhs=vTs[:, 0:NV],
                start=True, stop=False,
            )
            nc.tensor.matmul(
                h1p[:], lhsT=wst1[:, bass.ts(1, 128)], rhs=vTs[:, NV : 2 * NV],
                start=False, stop=False,
            )

            # ---- lstm transposes into K-major chunks 2..5 ----
            ident = wst1[0:NV, _C_IDENT : _C_IDENT + NV]
            vTl = sb.tile([128, 4 * NV], BF16, tag="vTl")
            for c in range(4):
                ptt = pst.tile([128, NV], BF16, tag="ptt")
                nc.tensor.transpose(ptt[:], nvL[:, 128 * c : 128 * (c + 1)], ident)
                nc.vector.tensor_copy(vTl[:, bass.ts(c, NV)], ptt[:])

            for c in range(4):
                nc.tensor.matmul(
                    h1p[:],
                    lhsT=wst1[:, bass.ts(2 + c, 128)],
                    rhs=vTl[:, bass.ts(c, NV)],
                    start=False, stop=(c == 3),
                )
            h1 = sb.tile([128, NV], BF16, tag="h1")
            nc.vector.tensor_scalar_max(h1[:], h1p[:], 0.0)

            # ---- stage 2: u = relu(w2.T @ h1 + b2)  [32, NV] ----
            up = ps.tile([HID, NV], F32, tag="small_p")
            nc.tensor.matmul(
                up[:], lhsT=wsm[:, _C_W2 : _C_W2 + HID], rhs=h1[:],
                start=True, stop=True,
            )
            nc.vector.tensor_scalar(
                out=u[0:HID, :], in0=up[:], scalar1=wfc[:, 0:1], scalar2=0.0,
                op0=ADD, op1=MAX,
            )

            # ---- stage 3: g1 = relu(ta1l.T @ u_last + ta1r.T @ u_0 + tab1)
            # cols 0:NB of u are node0, NB:NV are nodeL; tab1 rides row 32 of
            # the presummed block against u's ones row.
            g1p = ps.tile([128, NB], F32, tag="mid_p")
            nc.tensor.matmul(
                g1p[:], lhsT=wsm[0 : HID + 1, _C_TA1L : _C_TA1L + 128],
                rhs=u[:, NB:NV], start=True, stop=False,
            )
            nc.tensor.matmul(
                g1p[:], lhsT=wsm[0 : HID + 1, _C_TA1R : _C_TA1R + 128],
                rhs=u[:, 0:NB], start=False, stop=True,
            )
            g1 = sb.tile([128, NB], BF16, tag="g1")
            nc.vector.tensor_scalar_max(g1[:], g1p[:], 0.0)

            # ---- stage 4: g2 = relu(taw2.T @ g1 + tab2)  [32, NB] ----
            g2p = ps.tile([HID, NB], F32, tag="small_p")
            nc.tensor.matmul(
                g2p[:], lhsT=wsm[:, _C_TAW2 : _C_TAW2 + HID], rhs=g1[:],
                start=True, stop=True,
            )
            nc.vector.tensor_scalar(
                out=g2[0:HID, :], in0=g2p[:], scalar1=wfc[:, 1:2], scalar2=0.0,
                op0=ADD, op1=MAX,
            )

            # ---- stage 5: g3 = relu(ffw1.T @ g2)  [64, NB]; ffb1 in row 32
            g3p = ps.tile([2 * HID, NB], F32, tag="mid_p")
            nc.tensor.matmul(
                g3p[:], lhsT=wsm[0 : HID + 1, _C_FFW1 : _C_FFW1 + 2 * HID],
                rhs=g2[:], start=True, stop=True,
            )
            nc.vector.tensor_scalar_max(g3[0 : 2 * HID, :], g3p[:], 0.0)

            # ---- stage 6: g4 = relu(ffw2.T @ g3)  [32, NB]; ffb2 in row 64
            g4p = ps.tile([HID, NB], F32, tag="small_p")
            nc.tensor.matmul(
                g4p[:], lhsT=wsm[0 : 2 * HID + 1, _C_FFW2 : _C_FFW2 + HID],
                rhs=g3[:], start=True, stop=True,
            )
            nc.vector.tensor_scalar_max(g4[0:HID, :], g4p[:], 0.0)

            # ---- stage 7: out = tsw.T @ g4  [1, NB]; ts_b in row 32 ----
            op_ = ps.tile([1, NB], F32, tag="small_p")
            nc.tensor.matmul(
                op_[:], lhsT=wsm[0 : HID + 1, _C_TSW : _C_TSW + 1], rhs=g4[:],
                start=True, stop=True,
            )
            o = sb.tile([1, NB], F32, tag="o")
            nc.vector.tensor_copy(o[:], op_[:])
            nc.sync.dma_start(out_d[:], o[:])

    nc.finalize()
    return nc


def _pack_weights(inputs):
    def w(name, shape):
        return np.asarray(inputs[name], np.float32).reshape(shape)

    # w1p rows: [w1[0:240] | b1 | zeros(15) | w1[240:752]] -> 768
    w1 = w("node_w1", (NODE_DIM, 4 * HID))
    w1p = np.zeros((KDIM, 4 * HID), np.float32)
    w1p[0:240] = w1[0:240]
    w1p[240] = w("node_b1", (4 * HID,))
    w1p[SLOT_K:] = w1[240:]

    wst1 = np.zeros((128, _WST1_COLS), np.float32)
    for c in range(NCHUNK):
        wst1[:, 128 * c : 128 * (c + 1)] = w1p[128 * c : 128 * (c + 1), :]
    wst1[0:NV, _C_IDENT : _C_IDENT + NV] = np.eye(NV, dtype=np.float32)

    wsm = np.zeros((128, _WSM_COLS), np.float32)
    wsm[:, _C_W2 : _C_W2 + HID] = w("node_w2", (4 * HID, HID))
    taw1 = w("ta_w1", (6 * HID, 4 * HID))
    wsm[0:HID, _C_TA1L : _C_TA1L + 128] = taw1[0:HID]
    wsm[0:HID, _C_TA1R : _C_TA1R + 128] = taw1[HID:].reshape(5, HID, 128).sum(0)
    wsm[HID, _C_TA1R : _C_TA1R + 128] = w("ta_b1", (4 * HID,))
    wsm[:, _C_TAW2 : _C_TAW2 + HID] = w("ta_w2", (4 * HID, HID))
    wsm[0:HID, _C_FFW1 : _C_FFW1 + 2 * HID] = w("ff_w1", (HID, 2 * HID))
    wsm[HID, _C_FFW1 : _C_FFW1 + 2 * HID] = w("ff_b1", (2 * HID,))
    wsm[0 : 2 * HID, _C_FFW2 : _C_FFW2 + HID] = w("ff_w2", (2 * HID, HID))
    wsm[2 * HID, _C_FFW2 : _C_FFW2 + HID] = w("ff_b2", (HID,))
    wsm[0:HID, _C_TSW] = w("ts_w", (HID,))
    wsm[HID, _C_TSW] = w("ts_b", (1,))[0]

    wfc = np.zeros((HID, 2), np.float32)
    wfc[:, 0] = w("node_b2", (HID,))
    wfc[:, 1] = w("ta_b2", (HID,))
    return wst1.astype(NP_BF16), wsm.astype(NP_BF16), wfc


def make_in_maps(inputs):
    lstm = np.asarray(inputs["lstm_out_list"], np.float32).astype(NP_BF16)
    trees = np.asarray(inputs["trees"]).astype(np.int32)
    emb = np.asarray(inputs["emb"], np.float32)
    pe = _pos_encoding()

    wst1, wsm, wfc = _pack_weights(inputs)
    shared = {"wst1": wst1, "wsm": wsm, "wfc": wfc}
    in_maps = []
    for c in range(N_CORES):
        sl = slice(c * NB, (c + 1) * NB)
        # [64, 17]: rows 0:32 = node 0 of each batch, rows 32:64 = node N-1
        t2 = trees[sl][:, [0, N - 1], :].transpose(1, 0, 2).reshape(NV, 17)
        li = (t2[:, 16] + (np.arange(NV) % NB) * M).astype(np.int32)[:, None]
        # slot vectors, K-major bf16: vTs[p, 64k+q] = vs_ext[q, 128k+p]
        vs_ext = np.zeros((NV, SLOT_K), np.float32)
        vs_ext[:, 0:POS_DIM] = pe[t2[:, 0]]
        vs_ext[:, POS_DIM : 2 * POS_DIM] = pe[t2[:, 1]]
        vs_ext[:, 16:240] = emb[t2[:, 2:16]].reshape(NV, 14 * EMB_DIM)
        vs_ext[:, 240] = 1.0  # multiplies node_b1 (w1p row 240)
        vts = np.ascontiguousarray(
            vs_ext.astype(NP_BF16).reshape(NV, 2, 128).transpose(2, 1, 0)
            .reshape(128, 2 * NV)
        )
        in_maps.append(
            {
                "lstm": np.ascontiguousarray(lstm[sl].reshape(NB * M, LSTM_DIM)),
                "vts": vts,
                "li": li,
                **shared,
            }
        )
    return in_maps


_NC_CACHE = None


def run_on_hw(inputs, **kwargs):
    global _NC_CACHE
    if _NC_CACHE is None:
        _NC_CACHE = build_nc()
    in_maps = make_in_maps(inputs)
    return run_bass_kernel_spmd(
        _NC_CACHE, in_maps, core_ids=list(range(N_CORES)), **kwargs
    )


def kernel(**inputs) -> np.ndarray:
    res = run_on_hw(inputs)
    out = np.empty((B, 1), np.float32)
    for c in range(N_CORES):
        out[c * NB : (c + 1) * NB, 0] = res.results[c]["out"][0]
    return out


# revision 5
# speedup vs baseline: 1.1165x; 1.1165x over previous
"""Trainium2 Bass kernel for nn_Discriminator (attentional recent discriminator).

Math notes (derived from the module definition, hardcoded here):
  - The attention matmul result is deleted (torch sorts a size-1 dim, so the
    "top-5" indices are always 0); the output depends only on node_vec rows
    0 and N-1 of each batch element.
  - hidden_in rows 1..5 are all node_vec[:,0,:], so after the node MLP the
    five repeated u0 blocks contract against ta_w1 blocks 1..5; those blocks
    are PRE-SUMMED on the host, so stage 3 is 2 matmuls (last-block + summed).

Sharding: pure data parallel over batch, 32 batches/core on 8 cores.
Weights + tables replicated.

v5: the gpsimd ap_gather path (library load ~6us IRAM + ext-isa dispatch
quiesce put slot data at ~15us into the kernel) is gone.  The pe/emb
lookups index tiny replicated tables (200x8 / 200x16), so the host packs
the 64 needed slot vectors per core directly into a K-major bf16 tile
vTs [128, 2*64] (k-chunks 0,1 of the stage-1 contraction) shipped as an
input -- same class of input packing as the pre-summed ta_w1 blocks.
The large data-dependent gather (lstm rows, 512 wide out of a 6.5MB
per-core tensor) stays on device as an indirect DMA (64 descriptors,
one per out partition -- the HW-supported form), followed by 4 PE
transposes into K-major chunks 2..5.  Stage-1 slot matmuls issue as soon
as vTs+wst1 land (~8.5us), lstm matmuls right after the transposes.

Biases: node_b1 folded into K-row 240 (vTs carries a 1.0 there);
ta_b1/ff_b1/ff_b2/ts_b folded via an extra all-ones partition row on the
rhs tiles and a bias row in the packed weights; b2/ta_b2 applied with the
DVE tensor_scalar per-partition pointer.  All activations run on the DVE.

Precision: bf16 operands with fp32 PSUM accumulation.
"""

import ml_dtypes
import numpy as np

import concourse.bass as bass
import concourse.mybir as mybir
import concourse.tile as tile
from concourse import bacc
from concourse.bass import IndirectOffsetOnAxis
from concourse.bass_utils import run_bass_kernel_spmd

# problem constants (hardcoded per harness contract)
B, N, M = 256, 200, 200
EMB_DICT, EMB_DIM, POS_DIM, HID, LSTM_DIM, MAX_LEN, TOPK = 200, 16, 8, 32, 512, 200, 5
NODE_DIM = 2 * POS_DIM + 14 * EMB_DIM + LSTM_DIM  # 752

N_CORES = 8
NB = B // N_CORES  # 32 batches per core
NV = 2 * NB        # 64 node vectors per core (node 0 and node N-1)

SLOT_K = 256                  # pe0|pe1|emb*14|ones|pad, K-chunks 0,1
KDIM = SLOT_K + LSTM_DIM      # 768
NCHUNK = KDIM // 128          # 6

F32 = mybir.dt.float32
BF16 = mybir.dt.bfloat16
I32 = mybir.dt.int32
NP_BF16 = ml_dtypes.bfloat16

# wst1 (bf16) [128, 832]: stage-1 weight chunks at 128c, identity at 768
_C_IDENT = NCHUNK * 128  # 768
_WST1_COLS = 832

# wsm (bf16) [128, 417] column layout
_C_W2 = 0        # rows 0:128, node_w2
_C_TA1L = 32     # rows 0:33, ta_w1 block 0 (multiplies u_last), row 32 zero
_C_TA1R = 160    # rows 0:33, sum of ta_w1 blocks 1..5, row 32 = ta_b1
_C_TAW2 = 288    # rows 0:128
_C_FFW1 = 320    # rows 0:33, row 32 = ff_b1
_C_FFW2 = 384    # rows 0:65, row 64 = ff_b2
_C_TSW = 416     # rows 0:33, row 32 = ts_b
_WSM_COLS = 417

ADD = mybir.AluOpType.add
MAX = mybir.AluOpType.max


def _pos_encoding():
    pos = np.arange(MAX_LEN, dtype=np.float32)[:, None]
    div = np.exp(
        np.arange(0, POS_DIM, 2, dtype=np.float32) * (-np.log(10000.0) / POS_DIM)
    )
    pe = np.zeros((MAX_LEN, POS_DIM), np.float32)
    pe[:, 0::2] = np.sin(pos * div)
    pe[:, 1::2] = np.cos(pos * div)
    return pe


def build_nc():
    # Bacc (not plain Bass): its compile pass splits multi-wait sync into
    # InstEventSemaphore, which the walrus codegen requires (1 wait/inst).
    nc = bacc.Bacc(
        "TRN2",
        target_bir_lowering=False,
        debug=False,
        num_devices=N_CORES,
    )

    lstm = nc.dram_tensor("lstm", [NB * M, LSTM_DIM], BF16, kind="ExternalInput")
    vts_d = nc.dram_tensor("vts", [128, 2 * NV], BF16, kind="ExternalInput")
    li_d = nc.dram_tensor("li", [NV, 1], I32, kind="ExternalInput")
    wst1_d = nc.dram_tensor("wst1", [128, _WST1_COLS], BF16, kind="ExternalInput")
    wsm_d = nc.dram_tensor("wsm", [128, _WSM_COLS], BF16, kind="ExternalInput")
    wfc_d = nc.dram_tensor("wfc", [HID, 2], F32, kind="ExternalInput")

    out_d = nc.dram_tensor("out", [1, NB], F32, kind="ExternalOutput")

    with tile.TileContext(nc) as tc:
        with (
            tc.tile_pool(name="sb", bufs=1) as sb,
            tc.tile_pool(name="pst", bufs=3, space="PSUM") as pst,
            tc.tile_pool(name="ps", bufs=1, space="PSUM") as ps,
        ):
            # ---- input DMAs: li first (gates the lstm gather), then the
            # stage-1 operands, then the tail weights.
            li = sb.tile([NV, 1], I32, tag="li")
            nc.sync.dma_start(li[:], li_d[:])
            vTs = sb.tile([128, 2 * NV], BF16, tag="vTs")
            nc.sync.dma_start(vTs[:], vts_d[:])
            wst1 = sb.tile([128, _WST1_COLS], BF16, tag="wst1")
            nc.scalar.dma_start(wst1[:], wst1_d[:])
            wsm = sb.tile([128, _WSM_COLS], BF16, tag="wsm")
            nc.scalar.dma_start(wsm[:], wsm_d[:])
            wfc = sb.tile([HID, 2], F32, tag="wfc")
            nc.sync.dma_start(wfc[:], wfc_d[:])

            # ---- lstm gather: 64 descriptors on the dynamic queue ----
            nvL = sb.tile([NV, LSTM_DIM], BF16, tag="nvL")
            nc.gpsimd.indirect_dma_start(
                out=nvL[:], out_offset=None, in_=lstm[:],
                in_offset=IndirectOffsetOnAxis(ap=li[:, 0:1], axis=0),
            )

            # ---- ones rows for bias folding (off critical path) ----
            u = sb.tile([HID + 1, NV], BF16, tag="u")
            g2 = sb.tile([HID + 1, NB], BF16, tag="g2")
            g3 = sb.tile([2 * HID + 1, NB], BF16, tag="g3")
            g4 = sb.tile([HID + 1, NB], BF16, tag="g4")
            nc.vector.memset(u[HID : HID + 1, :], 1.0)
            nc.vector.memset(g2[HID : HID + 1, :], 1.0)
            nc.vector.memset(g3[2 * HID : 2 * HID + 1, :], 1.0)
            nc.vector.memset(g4[HID : HID + 1, :], 1.0)

            # ---- stage 1: h1 = relu(w1p.T @ v)  [128, NV]; b1 in K-row 240
            # slot chunks first (vTs lands early); lstm chunks after the
            # transposes fill vTl.
            h1p = ps.tile([128, NV], F32, tag="h1p")
            nc.tensor.matmul(
                h1p[:], lhsT=wst1[:, bass.ts(0, 128)], rhs=vTs[:, 0:NV],
                start=True, stop=False,
            )
            nc.tensor.matmul(
                h1p[:], lhsT=wst1[:, bass.ts(1, 128)], rhs=vTs[:, NV : 2 * NV],
                start=False, stop=False,
            )

            # ---- lstm transposes into K-major chunks 2..5 ----
            ident = wst1[0:NV, _C_IDENT : _C_IDENT + NV]
            vTl = sb.tile([128, 4 * NV], BF16, tag="vTl")
            for c in range(4):
                ptt = pst.tile([128, NV], BF16, tag="ptt")
                nc.tensor.transpose(ptt[:], nvL[:, 128 * c : 128 * (c + 1)], ident)
                nc.vector.tensor_copy(vTl[:, bass.ts(c, NV)], ptt[:])

            for c in range(4):
                nc.tensor.matmul(
                    h1p[:],
                    lhsT=wst1[:, bass.ts(2 + c, 128)],
                    rhs=vTl[:, bass.ts(c, NV)],
                    start=False, stop=(c == 3),
                )
            h1 = sb.tile([128, NV], BF16, tag="h1")
            nc.vector.tensor_scalar_max(h1[:], h1p[:], 0.0)

            # ---- stage 2: u = relu(w2.T @ h1 + b2)  [32, NV] ----
            up = ps.tile([HID, NV], F32, tag="small_p")
            nc.tensor.matmul(
                up[:], lhsT=wsm[:, _C_W2 : _C_W2 + HID], rhs=h1[:],
                start=True, stop=True,
            )
            nc.vector.tensor_scalar(
                out=u[0:HID, :], in0=up[:], scalar1=wfc[:, 0:1], scalar2=0.0,
                op0=ADD, op1=MAX,
            )

            # ---- stage 3: g1 = relu(ta1l.T @ u_last + ta1r.T @ u_0 + tab1)
            # cols 0:NB of u are node0, NB:NV are nodeL; tab1 rides row 32 of
            # the presummed block against u's ones row.
            g1p = ps.tile([128, NB], F32, tag="mid_p")
            nc.tensor.matmul(
                g1p[:], lhsT=wsm[0 : HID + 1, _C_TA1L : _C_TA1L + 128],
                rhs=u[:, NB:NV], start=True, stop=False,
            )
            nc.tensor.matmul(
                g1p[:], lhsT=wsm[0 : HID + 1, _C_TA1R : _C_TA1R + 128],
                rhs=u[:, 0:NB], start=False, stop=True,
            )
            g1 = sb.tile([128, NB], BF16, tag="g1")
            nc.vector.tensor_scalar_max(g1[:], g1p[:], 0.0)

            # ---- stage 4: g2 = relu(taw2.T @ g1 + tab2)  [32, NB] ----
            g2p = ps.tile([HID, NB], F32, tag="small_p")
            nc.tensor.matmul(
                g2p[:], lhsT=wsm[:, _C_TAW2 : _C_TAW2 + HID], rhs=g1[:],
                start=True, stop=True,
            )
            nc.vector.tensor_scalar(
                out=g2[0:HID, :], in0=g2p[:], scalar1=wfc[:, 1:2], scalar2=0.0,
                op0=ADD, op1=MAX,
            )

            # ---- stage 5: g3 = relu(ffw1.T @ g2)  [64, NB]; ffb1 in row 32
            g3p = ps.tile([2 * HID, NB], F32, tag="mid_p")
            nc.tensor.matmul(
                g3p[:], lhsT=wsm[0 : HID + 1, _C_FFW1 : _C_FFW1 + 2 * HID],
                rhs=g2[:], start=True, stop=True,
            )
            nc.vector.tensor_scalar_max(g3[0 : 2 * HID, :], g3p[:], 0.0)

            # ---- stage 6: g4 = relu(ffw2.T @ g3)  [32, NB]; ffb2 in row 64
            g4p = ps.tile([HID, NB], F32, tag="small_p")
            nc.tensor.matmul(
                g4p[:], lhsT=wsm[0 : 2 * HID + 1, _C_FFW2 : _C_FFW2 + HID],
                rhs=g3[:], start=True, stop=True,
            )
            nc.vector.tensor_scalar_max(g4[0:HID, :], g4p[:], 0.0)

            # ---- stage 7: out = tsw.T @ g4  [1, NB]; ts_b in row 32 ----
            op_ = ps.tile([1, NB], F32, tag="small_p")
            nc.tensor.matmul(
                op_[:], lhsT=wsm[0 : HID + 1, _C_TSW : _C_TSW + 1], rhs=g4[:],
                start=True, stop=True,
            )
            o = sb.tile([1, NB], F32, tag="o")
            nc.vector.tensor_copy(o[:], op_[:])
            # single_packet: the 128B result rides one packet/engine, so the
            # completion sem doesn't wait on 15 idle engines' sem-only
            # descriptors (observed ~2us straggler trickle on the tail).
            nc.sync.dma_start(out_d[:], o[:], single_packet=True)

    nc.finalize()
    return nc


def _pack_weights(inputs):
    def w(name, shape):
        return np.asarray(inputs[name], np.float32).reshape(shape)

    # w1p rows: [w1[0:240] | b1 | zeros(15) | w1[240:752]] -> 768
    w1 = w("node_w1", (NODE_DIM, 4 * HID))
    w1p = np.zeros((KDIM, 4 * HID), np.float32)
    w1p[0:240] = w1[0:240]
    w1p[240] = w("node_b1", (4 * HID,))
    w1p[SLOT_K:] = w1[240:]

    wst1 = np.zeros((128, _WST1_COLS), np.float32)
    for c in range(NCHUNK):
        wst1[:, 128 * c : 128 * (c + 1)] = w1p[128 * c : 128 * (c + 1), :]
    wst1[0:NV, _C_IDENT : _C_IDENT + NV] = np.eye(NV, dtype=np.float32)

    wsm = np.zeros((128, _WSM_COLS), np.float32)
    wsm[:, _C_W2 : _C_W2 + HID] = w("node_w2", (4 * HID, HID))
    taw1 = w("ta_w1", (6 * HID, 4 * HID))
    wsm[0:HID, _C_TA1L : _C_TA1L + 128] = taw1[0:HID]
    wsm[0:HID, _C_TA1R : _C_TA1R + 128] = taw1[HID:].reshape(5, HID, 128).sum(0)
    wsm[HID, _C_TA1R : _C_TA1R + 128] = w("ta_b1", (4 * HID,))
    wsm[:, _C_TAW2 : _C_TAW2 + HID] = w("ta_w2", (4 * HID, HID))
    wsm[0:HID, _C_FFW1 : _C_FFW1 + 2 * HID] = w("ff_w1", (HID, 2 * HID))
    wsm[HID, _C_FFW1 : _C_FFW1 + 2 * HID] = w("ff_b1", (2 * HID,))
    wsm[0 : 2 * HID, _C_FFW2 : _C_FFW2 + HID] = w("ff_w2", (2 * HID, HID))
    wsm[2 * HID, _C_FFW2 : _C_FFW2 + HID] = w("ff_b2", (HID,))
    wsm[0:HID, _C_TSW] = w("ts_w", (HID,))
    wsm[HID, _C_TSW] = w("ts_b", (1,))[0]

    wfc = np.zeros((HID, 2), np.float32)
    wfc[:, 0] = w("node_b2", (HID,))
    wfc[:, 1] = w("ta_b2", (HID,))
    return wst1.astype(NP_BF16), wsm.astype(NP_BF16), wfc


def make_in_maps(inputs):
    lstm = np.asarray(inputs["lstm_out_list"], np.float32).astype(NP_BF16)
    trees = np.asarray(inputs["trees"]).astype(np.int32)
    emb = np.asarray(inputs["emb"], np.float32)
    pe = _pos_encoding()

    wst1, wsm, wfc = _pack_weights(inputs)
    shared = {"wst1": wst1, "wsm": wsm, "wfc": wfc}
    in_maps = []
    for c in range(N_CORES):
        sl = slice(c * NB, (c + 1) * NB)
        # [64, 17]: rows 0:32 = node 0 of each batch, rows 32:64 = node N-1
        t2 = trees[sl][:, [0, N - 1], :].transpose(1, 0, 2).reshape(NV, 17)
        li = (t2[:, 16] + (np.arange(NV) % NB) * M).astype(np.int32)[:, None]
        # slot vectors, K-major bf16: vTs[p, 64k+q] = vs_ext[q, 128k+p]
        vs_ext = np.zeros((NV, SLOT_K), np.float32)
        vs_ext[:, 0:POS_DIM] = pe[t2[:, 0]]
        vs_ext[:, POS_DIM : 2 * POS_DIM] = pe[t2[:, 1]]
        vs_ext[:, 16:240] = emb[t2[:, 2:16]].reshape(NV, 14 * EMB_DIM)
        vs_ext[:, 240] = 1.0  # multiplies node_b1 (w1p row 240)
        vts = np.ascontiguousarray(
            vs_ext.astype(NP_BF16).reshape(NV, 2, 128).transpose(2, 1, 0)
            .reshape(128, 2 * NV)
        )
        in_maps.append(
            {
                "lstm": np.ascontiguousarray(lstm[sl].reshape(NB * M, LSTM_DIM)),
                "vts": vts,
                "li": li,
                **shared,
            }
        )
    return in_maps


_NC_CACHE = None


def run_on_hw(inputs, **kwargs):
    global _NC_CACHE
    if _NC_CACHE is None:
        _NC_CACHE = build_nc()
    in_maps = make_in_maps(inputs)
    return run_bass_kernel_spmd(
        _NC_CACHE, in_maps, core_ids=list(range(N_CORES)), **kwargs
    )


def kernel(**inputs) -> np.ndarray:
    res = run_on_hw(inputs)
    out = np.empty((B, 1), np.float32)
    for c in range(N_CORES):
        out[c * NB : (c + 1) * NB, 0] = res.results[c]["out"][0]
    return out
